# revision 16
# baseline (speedup 1.0000x reference)
"""Trainium2 Bass kernel for pairwise-channel-correlation pooling.

reference math (per sample, f: [256 ch, 25 pos]):
    G[i,j]  = sum_p (|f_ip + f_jp| - |f_ip - f_jp|)/2 * exp(T)
            = exp(T) * sum_p sign(f_ip) sign(f_jp) min(|f_ip|, |f_jp|)
    out     = G - rowmean(G) - colmean(G), then triu(row-major).

Device strategy (per core, 8 samples, pure data parallelism):
  - tiles of [(5 i-chans x 25 pos) = 125 partitions, 256 j] where
        t[q=(il,p), j] = clip(f_jp, -|f_ip|, +|f_ip|)
                       = sign(f_jp) * min(|f_ip|,|f_jp|)
    is ONE dual-op tensor_scalar (max, min) on the vector engine.
  - a stationary +-1 matrix S_g = sign(a)*mask_g on the tensor engine
    applies sign(f_ip), reduces over p, scatters the 5 channel rows into
    the PSUM block, and accumulates a column-sum row (exact: weights are
    only 0/+-1, accumulation fp32 in PSUM; fp32r streams 1 col/cycle).
  - row/col means + exp(T) scale applied from PSUM; host gathers triu.
"""

import sys

sys.path.insert(0, "/opt/trn_rl_repo")

import numpy as np

import concourse.bass as bass
import concourse.tile as tile
from concourse import bacc, mybir
from concourse import bass_utils

F32 = mybir.dt.float32
F32R = mybir.dt.float32r

B, D, H, W = 64, 256, 5, 5
HW = H * W  # 25
NCORES = 8
SPC = B // NCORES  # samples per core
NROW = 125  # K rows per tile: 5 chans x 25 pos
NT = 52  # tiles per sample (51 full + 1 tail)
NV = 25  # distinct mask variants
CS_ROW = 126  # psum row accumulating column sums

_BUILT = {}
_BUILT_LOOP = {}


def _build_masks() -> np.ndarray:
    """[125, NV*128] f32; variant v maps local row (il,p) -> col 5v+il,
    plus col CS_ROW=1 (column-sum accumulator row)."""
    m = np.zeros((NROW, NV, 128), dtype=np.float32)
    for v in range(NV):
        for il in range(5):
            m[il * 25 : (il + 1) * 25, v, 5 * v + il] = 1.0
    m[:, :, CS_ROW] = 1.0
    return np.ascontiguousarray(m.reshape(NROW, NV * 128))


def _build_kernel(loop_k=None):
    cache = _BUILT if loop_k is None else _BUILT_LOOP
    if "nc" in cache and cache.get("loop_k") == loop_k:
        return cache
    Alu = mybir.AluOpType
    Act = mybir.ActivationFunctionType

    nc = bacc.Bacc("TRN2", target_bir_lowering=False, debug=False,
                   num_devices=NCORES)
    b2_d = nc.dram_tensor("b2", [SPC, NROW, D], F32, kind="ExternalInput")
    a_d = nc.dram_tensor("a", [SPC, NROW, NT], F32, kind="ExternalInput")
    temp_d = nc.dram_tensor("temp", [1, 1], F32, kind="ExternalInput")
    out_d = nc.dram_tensor("out", [SPC, D, D], F32, kind="ExternalOutput")
    masks_d = nc.inline_tensor(_build_masks(), name="masks")

    with tile.TileContext(nc) as tc:
        with (
            tc.tile_pool(name="const", bufs=1) as cpool,
            tc.tile_pool(name="inp", bufs=2) as ipool,
            tc.tile_pool(name="acol", bufs=2) as apool,
            tc.tile_pool(name="sg", bufs=4) as spool,
            tc.tile_pool(name="tg", bufs=4) as tpool,
            tc.tile_pool(name="post", bufs=3) as opool,
            tc.tile_pool(name="psum", bufs=1, space="PSUM") as pspool,
        ):
            # ---- constants ----
            mask_sb = cpool.tile([NROW, NV * 128], F32)
            nc.sync.dma_start(mask_sb[:], masks_d.ap())
            tsb = cpool.tile([1, 1], F32)
            nc.sync.dma_start(tsb[:], temp_d.ap())
            expT = cpool.tile([1, 1], F32)
            nc.scalar.activation(expT[:], tsb[:], Act.Exp)
            expT_col = cpool.tile([128, 1], F32)
            nc.gpsimd.partition_broadcast(expT_col[:], expT[:])

            import contextlib

            loop_cm = (
                tc.For_i(0, loop_k, 1) if loop_k is not None
                else contextlib.nullcontext()
            )
            with loop_cm:
                _emit_body(nc, tc, mask_sb, expT, expT_col,
                           b2_d, a_d, out_d,
                           ipool, apool, spool, tpool, opool, pspool)

    nc.compile()
    cache.clear()
    cache["nc"] = nc
    cache["loop_k"] = loop_k
    return cache


def _emit_body(nc, tc, mask_sb, expT, expT_col, b2_d, a_d, out_d,
               ipool, apool, spool, tpool, opool, pspool):
    Alu = mybir.AluOpType
    Act = mybir.ActivationFunctionType
    if True:
        if True:
            for s in range(SPC):
                b2 = ipool.tile([NROW, D], F32)
                nc.sync.dma_start(b2[:], b2_d.ap()[s])
                a = ipool.tile([NROW, NT], F32)
                nc.sync.dma_start(a[:], a_d.ap()[s])

                absA = apool.tile([NROW, NT], F32)
                nc.scalar.activation(absA[:], a[:], Act.Abs)
                negA = apool.tile([NROW, NT], F32)
                nc.vector.tensor_scalar_mul(negA[:], absA[:], -1.0)
                signA = apool.tile([NROW, NT], F32)
                nc.scalar.sign(signA[:], a[:])
                signA_r = apool.tile([NROW, NT], F32R)
                nc.vector.tensor_copy(signA_r[:], signA[:])

                psums = [
                    pspool.tile([128, D], F32, tag="ps", name=f"ps_{s}_{b}",
                                bufs=6)
                    for b in range(3)
                ]
                psum_cs = pspool.tile([1, D], F32, tag="psc", bufs=2)

                for g in range(NT):
                    blk = g // 25 if g < 50 else 2
                    v = g % 25
                    first = (g % 25 == 0) if g < 50 else (g == 50)
                    last = (g % 25 == 24) if g < 50 else (g == 51)

                    Sg = spool.tile([NROW, 128], F32R)
                    nc.vector.tensor_scalar(
                        Sg[:], mask_sb[:, v * 128 : (v + 1) * 128],
                        signA[:, g : g + 1], None, Alu.mult,
                    )
                    tg = tpool.tile([NROW, D], F32R)
                    nc.vector.tensor_scalar(
                        tg[:], b2[:],
                        negA[:, g : g + 1], absA[:, g : g + 1],
                        Alu.max, Alu.min,
                    )
                    nc.tensor.matmul(
                        psums[blk][:, :],
                        Sg[:],
                        tg[:],
                        start=first, stop=last,
                    )
                    # column-sum accumulator: cs[j] += sum_q sign(a_q)*t[q,j]
                    nc.tensor.matmul(
                        psum_cs[:, :],
                        signA_r[:, g : g + 1],
                        tg[:],
                        start=(g == 0), stop=(g == NT - 1),
                    )

                # cs_scaled = colsum * expT / 256
                cs_s = opool.tile([1, D], F32, tag="cs")
                nc.vector.tensor_scalar(
                    cs_s[:], psum_cs[:, :], expT[:], 1.0 / D, Alu.mult, Alu.mult,
                )
                cs_bc = opool.tile([128, D], F32, tag="csb")
                nc.gpsimd.partition_broadcast(cs_bc[:], cs_s[:])

                row0 = 0
                for blk in range(3):
                    rows = NROW if blk < 2 else 6
                    ps = psums[blk]
                    rsum = opool.tile([NROW, 1], F32, tag="rs")
                    nc.vector.tensor_reduce(
                        rsum[:rows], ps[:rows, :], mybir.AxisListType.X, Alu.add,
                    )
                    rmean = opool.tile([NROW, 1], F32, tag="rm")
                    nc.vector.tensor_scalar_mul(rmean[:rows], rsum[:rows], 1.0 / D)
                    o1 = opool.tile([NROW, D], F32, tag="o1")
                    nc.vector.tensor_scalar(
                        o1[:rows], ps[:rows, :],
                        rmean[:rows], expT_col[:rows],
                        Alu.subtract, Alu.mult,
                    )
                    o2 = opool.tile([NROW, D], F32, tag="o2")
                    nc.vector.tensor_sub(o2[:rows], o1[:rows], cs_bc[:rows])
                    nc.sync.dma_start(out_d.ap()[s, row0 : row0 + rows, :], o2[:rows])
                    row0 += rows


def _prep_core_inputs(f_core: np.ndarray, temp: np.ndarray) -> dict:
    """f_core: [SPC, D, H, W] -> relayout for the device kernel."""
    fr = f_core.reshape(SPC, D, HW).astype(np.float32)
    # b2[s] = f^T tiled 5x along partitions: [(il,p), j] = f[j, p]
    ft = np.transpose(fr, (0, 2, 1))  # [SPC, 25, 256]
    b2 = np.tile(ft, (1, 5, 1))  # [SPC, 125, 256]
    # a[s]: col g = f.flat[125g:125(g+1)], zero-padded to 52*125
    flat = fr.reshape(SPC, D * HW)
    pad = np.zeros((SPC, NT * NROW), dtype=np.float32)
    pad[:, : D * HW] = flat
    a = np.transpose(pad.reshape(SPC, NT, NROW), (0, 2, 1))  # [SPC, 125, 52]
    return {
        "b2": np.ascontiguousarray(b2),
        "a": np.ascontiguousarray(a),
        "temp": temp.astype(np.float32).reshape(1, 1),
    }


_IU, _JU = np.triu_indices(D)


def kernel(feat_map: np.ndarray, temperature: np.ndarray) -> np.ndarray:
    built = _build_kernel()
    nc = built["nc"]
    in_maps = [
        _prep_core_inputs(feat_map[c * SPC : (c + 1) * SPC], temperature)
        for c in range(NCORES)
    ]
    res = bass_utils.run_bass_kernel_spmd(
        nc, in_maps, core_ids=list(range(NCORES))
    )
    full = np.concatenate([res.results[c]["out"] for c in range(NCORES)], axis=0)
    return np.ascontiguousarray(full[:, _IU, _JU])
